# revision 1
# baseline (speedup 1.0000x reference)
"""Trainium2 Bass kernel for the ContinuousGRULayer problem.

Data-parallel over batch: 8 cores, 64 batch rows each. The T=512 time
recurrence runs locally per core with the hidden state kept in transposed
layout [H partitions, B free] so every recurrent matmul is a native
lhsT.T @ rhs with no per-step transposes.

Per step (all elementwise tiles live on partitions 0:64, lane-aligned):
  flow layer l:  ps_g = wtt_g (x) tt  (+accumulate)  W_g @ h   for g in {r,z}
                 sig_g = sigmoid(ps_g + b_g)           (ACT, bias fused)
                 u = tanh(W_u @ (sig_r * h) + wtt_u (x) tt + b_u)   [beta in W_u]
                 h += ((alpha*sig_z) * tanh(tw (x) tt)) * (u - h)
  GRU cell:      x-side matmuls accumulate into PSUM off the critical path;
                 n-gate uses fused scalar_tensor_tensor:
                 rhn = (h_n + b_hn)*r ; s = (i_n + b_in) + rhn ; n = tanh(s)
                 h = n + z*(h - n)

tanh(tw (x) tt) is precomputed on PE+ACT in 8-step chunks (rank-1 matmul
tw (x) tt into PSUM, tanh to SBUF), overlapped with the recurrence.

All weights/biases ride in one packed [64, WB_COLS] tensor (single DMA) to
keep per-instruction semaphore wait counts low.
"""

import numpy as np

import concourse.bass as bass
import concourse.bacc as bacc
import concourse.mybir as mybir
from concourse.tile import TileContext
from concourse.bass_utils import run_bass_kernel_spmd

B, T, D, H, L = 512, 512, 32, 64, 2
NCORES = 8
BL = B // NCORES  # 64 batch rows per core
ALPHA, BETA = 2.0 / 5.0, 4.0 / 5.0
FP = mybir.dt.float32
AF = mybir.ActivationFunctionType
OP = mybir.AluOpType

# packed weight layout: name -> (row_count, col_offset, col_width)
_W64 = ["whr0", "whz0", "whu0", "whr1", "whz1", "whu1", "ggr", "ggz", "ggn"]
_W32 = ["gxr", "gxz", "gxn"]
_W1 = ["wtr0", "wtz0", "wtu0", "tw0", "wtr1", "wtz1", "wtu1", "tw1"]
_WB = ["br0", "bz0", "bu0", "br1", "bz1", "bu1", "gbr", "gbz", "gbhn", "gbin"]


def _wb_layout():
    lay, off = {}, 0
    for n in _W64:
        lay[n] = (64, off, 64)
        off += 64
    for n in _W32:
        lay[n] = (32, off, 64)
        off += 64
    for n in _W1:
        lay[n] = (1, off, 64)
        off += 64
    for n in _WB:
        lay[n] = (64, off, 1)
        off += 1
    return lay, off


_WLAY, WB_COLS = _wb_layout()


def _build(t_steps=T, reps=1):
    assert t_steps % 8 == 0
    nchunks = t_steps // 8
    nc = bacc.Bacc("TRN2", debug=False, enable_asserts=False)

    xp = nc.dram_tensor("xp", [D, t_steps * BL], FP, kind="ExternalInput").ap()
    ttf = nc.dram_tensor("ttf", [nchunks, 8 * BL], FP, kind="ExternalInput").ap()
    wb = nc.dram_tensor("wb", [64, WB_COLS], FP, kind="ExternalInput").ap()
    out = nc.dram_tensor("out", [t_steps, H, BL], FP, kind="ExternalOutput").ap()

    with TileContext(nc) as tc:
        with (
            tc.tile_pool(name="const", bufs=1) as cpool,
            tc.tile_pool(name="ps", bufs=6, space="PSUM") as pspool,
            tc.tile_pool(name="taups", bufs=2, space="PSUM") as taupspool,
            tc.tile_pool(name="sb", bufs=3) as sbpool,
            tc.tile_pool(name="taopool", bufs=4) as taupool,
        ):
            x_sb = cpool.tile([D, t_steps * BL], FP, tag="x", name="x_sb")
            nc.sync.dma_start(out=x_sb[:], in_=xp[:])
            wb_sb = cpool.tile([64, WB_COLS], FP, tag="wb", name="wb_sb")
            nc.sync.dma_start(out=wb_sb[:], in_=wb[:])

            def W(name):
                r, o, w = _WLAY[name]
                return wb_sb[0:r, o:o + w]

            for _rep in range(reps):
              h_cur = sbpool.tile([H, BL], FP, tag="h", bufs=4, name="h0")
              nc.vector.memset(h_cur[:], 0.0)

              tau = [None, None]
              ttchunk = None
              for t in range(t_steps):
                  if t % 8 == 0:
                      c = t // 8
                      # stage this chunk's tt values at partition 0 for matmuls
                      ttchunk = sbpool.tile([1, 8 * BL], FP, tag="ttc", bufs=3,
                                            name="ttc")
                      nc.sync.dma_start(out=ttchunk[:], in_=ttf[c:c + 1, :])
                      for l in range(L):
                          tps = taupspool.tile([H, 8 * BL], FP, tag="taups",
                                               name="taups")
                          nc.tensor.matmul(tps[:], W(f"tw{l}"), ttchunk[:],
                                           start=True, stop=True)
                          tau_t = taupool.tile([H, 8 * BL], FP, tag=f"tau{l}",
                                               name=f"tau{l}")
                          nc.scalar.activation(tau_t[:], tps[:], AF.Tanh)
                          nc.vector.tensor_scalar_mul(tau_t[:], tau_t[:], ALPHA)
                          tau[l] = tau_t
                  ttrow = ttchunk[0:1, (t % 8) * BL:(t % 8 + 1) * BL]
                  toff = (t % 8) * BL

                  # ---- flow layers (the hiddens output is the post-flow state)
                  for l in range(L):
                      ps_r = pspool.tile([H, BL], FP, tag="ps", name="ps_r")
                      nc.tensor.matmul(ps_r[:], W(f"wtr{l}"), ttrow,
                                       start=True, stop=False)
                      nc.tensor.matmul(ps_r[:], W(f"whr{l}"), h_cur[:],
                                       start=False, stop=True)
                      ps_z = pspool.tile([H, BL], FP, tag="ps", name="ps_z")
                      nc.tensor.matmul(ps_z[:], W(f"wtz{l}"), ttrow,
                                       start=True, stop=False)
                      nc.tensor.matmul(ps_z[:], W(f"whz{l}"), h_cur[:],
                                       start=False, stop=True)
                      sr = sbpool.tile([H, BL], FP, tag="sr", name="sr")
                      nc.scalar.activation(sr[:], ps_r[:], AF.Sigmoid,
                                           bias=W(f"br{l}"))
                      sz = sbpool.tile([H, BL], FP, tag="sz", name="sz")
                      nc.scalar.activation(sz[:], ps_z[:], AF.Sigmoid,
                                           bias=W(f"bz{l}"))
                      # g = (alpha*sig_z) * tanh(tw (x) tt): off the critical path
                      g = sbpool.tile([H, BL], FP, tag="g", name="g")
                      nc.gpsimd.tensor_mul(g[:], sz[:], tau[l][:, toff:toff + BL])
                      rh = sbpool.tile([H, BL], FP, tag="rh", name="rh")
                      nc.vector.tensor_mul(rh[:], sr[:], h_cur[:])
                      ps_u = pspool.tile([H, BL], FP, tag="ps", name="ps_u")
                      nc.tensor.matmul(ps_u[:], W(f"wtu{l}"), ttrow,
                                       start=True, stop=False)
                      nc.tensor.matmul(ps_u[:], W(f"whu{l}"), rh[:],
                                       start=False, stop=True)
                      u = sbpool.tile([H, BL], FP, tag="u", name="u")
                      nc.scalar.activation(u[:], ps_u[:], AF.Tanh,
                                           bias=W(f"bu{l}"))
                      dd = sbpool.tile([H, BL], FP, tag="dd", name="dd")
                      nc.vector.tensor_sub(dd[:], u[:], h_cur[:])
                      ee = sbpool.tile([H, BL], FP, tag="ee", name="ee")
                      nc.vector.tensor_mul(ee[:], g[:], dd[:])
                      h_new = sbpool.tile([H, BL], FP, tag="h", bufs=4,
                                          name="hf")
                      nc.vector.tensor_add(h_new[:], h_cur[:], ee[:])
                      h_cur = h_new

                  nc.sync.dma_start(out=out[t], in_=h_cur[:])

                  # ---- GRU cell (next step's carry; not needed after last step)
                  if t < t_steps - 1:
                      xs = x_sb[:, t * BL:(t + 1) * BL]
                      ps_gr = pspool.tile([H, BL], FP, tag="ps", name="ps_gr")
                      nc.tensor.matmul(ps_gr[:], W("gxr"), xs,
                                       start=True, stop=False)
                      nc.tensor.matmul(ps_gr[:], W("ggr"), h_cur[:],
                                       start=False, stop=True)
                      ps_gz = pspool.tile([H, BL], FP, tag="ps", name="ps_gz")
                      nc.tensor.matmul(ps_gz[:], W("gxz"), xs,
                                       start=True, stop=False)
                      nc.tensor.matmul(ps_gz[:], W("ggz"), h_cur[:],
                                       start=False, stop=True)
                      gsr = sbpool.tile([H, BL], FP, tag="sr", name="gsr")
                      nc.scalar.activation(gsr[:], ps_gr[:], AF.Sigmoid,
                                           bias=W("gbr"))
                      gsz = sbpool.tile([H, BL], FP, tag="sz", name="gsz")
                      nc.scalar.activation(gsz[:], ps_gz[:], AF.Sigmoid,
                                           bias=W("gbz"))
                      ps_in = pspool.tile([H, BL], FP, tag="ps", name="ps_in")
                      nc.tensor.matmul(ps_in[:], W("gxn"), xs,
                                       start=True, stop=True)
                      ps_hn = pspool.tile([H, BL], FP, tag="ps", name="ps_hn")
                      nc.tensor.matmul(ps_hn[:], W("ggn"), h_cur[:],
                                       start=True, stop=True)
                      rhn = sbpool.tile([H, BL], FP, tag="rhn", name="rhn")
                      nc.vector.scalar_tensor_tensor(
                          rhn[:], ps_hn[:], W("gbhn"), gsr[:],
                          op0=OP.add, op1=OP.mult)
                      s = sbpool.tile([H, BL], FP, tag="s", name="s")
                      nc.vector.scalar_tensor_tensor(
                          s[:], ps_in[:], W("gbin"), rhn[:],
                          op0=OP.add, op1=OP.add)
                      n_t = sbpool.tile([H, BL], FP, tag="n", name="n")
                      nc.scalar.activation(n_t[:], s[:], AF.Tanh)
                      dn = sbpool.tile([H, BL], FP, tag="dd", name="dn")
                      nc.vector.tensor_sub(dn[:], h_cur[:], n_t[:])
                      en = sbpool.tile([H, BL], FP, tag="ee", name="en")
                      nc.vector.tensor_mul(en[:], gsz[:], dn[:])
                      h_new = sbpool.tile([H, BL], FP, tag="h", bufs=4,
                                          name="hg")
                      nc.vector.tensor_add(h_new[:], n_t[:], en[:])
                      h_cur = h_new
    nc.compile()
    return nc


_NC_CACHE = {}


def _get_nc(t_steps=T, reps=1):
    key = (t_steps, reps)
    if key not in _NC_CACHE:
        _NC_CACHE[key] = _build(t_steps, reps)
    return _NC_CACHE[key]


def _pack_weights(inputs):
    f32 = lambda a: np.ascontiguousarray(np.asarray(a, np.float32))
    W_hr, b_hr = f32(inputs["flow_W_hr"]), f32(inputs["flow_b_hr"])
    W_hz, b_hz = f32(inputs["flow_W_hz"]), f32(inputs["flow_b_hz"])
    W_hh, b_hh = f32(inputs["flow_W_hh"]), f32(inputs["flow_b_hh"])
    tw = f32(inputs["flow_tw"])
    gW_ih, gW_hh = f32(inputs["gru_W_ih"]), f32(inputs["gru_W_hh"])
    gb_ih, gb_hh = f32(inputs["gru_b_ih"]), f32(inputs["gru_b_hh"])
    m = {}
    for l in range(L):
        m[f"whr{l}"] = W_hr[l][:, :H].T
        m[f"whz{l}"] = W_hz[l][:, :H].T
        m[f"wtr{l}"] = W_hr[l][:, H][None]
        m[f"wtz{l}"] = W_hz[l][:, H][None]
        m[f"br{l}"] = b_hr[l][:, None]
        m[f"bz{l}"] = b_hz[l][:, None]
        m[f"whu{l}"] = (BETA * W_hh[l][:, :H]).T
        m[f"wtu{l}"] = W_hh[l][:, H][None]
        m[f"bu{l}"] = b_hh[l][:, None]
        m[f"tw{l}"] = tw[l][None]
    m["ggr"] = gW_hh[0:H].T
    m["ggz"] = gW_hh[H:2 * H].T
    m["ggn"] = gW_hh[2 * H:].T
    m["gxr"] = gW_ih[0:H].T
    m["gxz"] = gW_ih[H:2 * H].T
    m["gxn"] = gW_ih[2 * H:].T
    m["gbr"] = (gb_ih + gb_hh)[0:H][:, None]
    m["gbz"] = (gb_ih + gb_hh)[H:2 * H][:, None]
    m["gbhn"] = gb_hh[2 * H:][:, None]
    m["gbin"] = gb_ih[2 * H:][:, None]
    wbarr = np.zeros((64, WB_COLS), np.float32)
    for name, (r, o, w) in _WLAY.items():
        arr = m[name]
        assert arr.shape == (r, w), (name, arr.shape, (r, w))
        wbarr[0:r, o:o + w] = arr
    return wbarr


def make_in_maps(inputs, t_steps=T):
    x = np.asarray(inputs["x"], np.float32)
    t = np.asarray(inputs["t"], np.float32)
    wbarr = _pack_weights(inputs)
    in_maps = []
    for c in range(NCORES):
        bs, be = c * BL, (c + 1) * BL
        xc = x[bs:be, :t_steps].transpose(2, 1, 0).reshape(D, t_steps * BL)
        ttc = t[bs:be, :t_steps, 0].T.reshape(t_steps // 8, 8 * BL)
        in_maps.append({"xp": np.ascontiguousarray(xc),
                        "ttf": np.ascontiguousarray(ttc), "wb": wbarr})
    return in_maps


def run(inputs, t_steps=T, reps=1, **kw):
    nc = _get_nc(t_steps, reps)
    res = run_bass_kernel_spmd(nc, make_in_maps(inputs, t_steps),
                               core_ids=list(range(NCORES)), **kw)
    outs = [res.results[c]["out"].reshape(t_steps, H, BL).transpose(2, 0, 1)
            for c in range(NCORES)]
    return np.concatenate(outs, 0).astype(np.float32), res


def kernel(**inputs):
    o, _ = run(inputs)
    return o

